# revision 1
# baseline (speedup 1.0000x reference)
"""Trainium2 Bass kernel for nn_AtLocPlusCriterion_VO.

loss = exp(-srx)*mean|vo_t - tg_t| + srx + exp(-srq)*mean|vo_q - tg_q| + srq
with vo = calc_vo_logq(pred[:-1], pred[1:]) (relative SE(3) pose, log-quaternion).

Sequence-parallel across 8 NeuronCores (1-row halo per shard). Inputs are
resharded host-side into component-major (SoA) planes so every on-device
access is contiguous. Per core: 1956 pairs per SBUF partition, 3 tiles of
652. Row phase (qexp via quarter-angle Sin LUT + Ln/Exp roots) in f32;
pair phase (rotation by two cross products, quaternion product, log map via
arctan) in bf16 on VectorE with unary work offloaded to ScalarE. Mean-L1
reduces through fused per-partition accumulators; host sums 8x[128,2].
"""
import os
import numpy as np

N_CORES = 8
T_FULL = 2_000_000
NPAIRS = T_FULL - 1          # 1_999_999
D = 1956                     # pairs per partition per core
C = 652                      # pairs per tile (3 tiles)
NT = 3
R = C + 1
PPC = 128 * D                # 250_368 pairs per core
PAIRS_PAD = N_CORES * PPC    # 2_002_944
ROWS_PAD = PAIRS_PAD + 1

PRED_LEN = 6 * (PPC + 1)
TARG_LEN = 6 * PPC

LN4 = float(np.log(4.0))
LN4SQ2 = float(np.log(4.0 * np.sqrt(2.0)))   # i2n carries 4*sqrt2
PI2 = float(np.pi / 2.0)
SQ2 = float(np.sqrt(2.0))

_BUILT = {}


def _patch_act_tables():
    import concourse.bacc as bacc_mod
    import concourse.hw_specs as hw

    if getattr(bacc_mod, "_vo_tables_patched", False):
        return
    orig = hw.get_activation_tables

    def steered(arch, _orig=orig):
        from concourse import mybir as _mb
        AF = _mb.ActivationFunctionType
        t = {k: set(v) for k, v in _orig(arch).items()}
        # Keep all 24 entries (act_func_set_id indexes the original list);
        # drop ln/exp/arctan from the earlier sets so the table-load pass
        # resolves them to natural_log_exp_and_others / trig_and_small.
        t.get("natural_log", set()).discard(AF.Ln)
        t.get("exp_and_others", set()).discard(AF.Exp)
        t.get("sigmoid_and_others", set()).discard(AF.Arctan)
        return t

    bacc_mod.get_activation_tables = steered
    bacc_mod._vo_tables_patched = True


def _build():
    from concourse import bacc, tile, mybir
    from concourse.ap import AP
    from concourse.bass import _add_dep_helper

    _patch_act_tables()

    f32, bf16 = mybir.dt.float32, mybir.dt.bfloat16
    OP = mybir.AluOpType
    AF = mybir.ActivationFunctionType

    nc = bacc.Bacc("TRN2", target_bir_lowering=False, debug=False,
                   num_devices=N_CORES)
    pred_h = nc.declare_dram_parameter("pred", [PRED_LEN], f32, isOutput=False)
    targ_h = nc.declare_dram_parameter("targ", [TARG_LEN], f32, isOutput=False)
    out_h = nc.declare_dram_parameter("out", [128, 2], f32, isOutput=True)

    for v in (1e-16, -LN4, LN4SQ2, PI2):
        v = float(v)
        if (f32, v) not in nc.const_aps.aps:
            t = nc.alloc_sbuf_tensor(f"uconst-{v}", [128, 1], f32)
            nc.gpsimd.memset(t.ap(), v)
            nc.const_aps.aps[(f32, v)] = t.ap()
    nc.all_engine_barrier()

    PL_P = PPC + 1   # pred plane length
    PL_T = PPC       # targ plane length

    def sb(tile_, off, dims):
        base = tile_[:, :]
        return AP(base.tensor, base.offset + off,
                  [[base.ap.to_list()[0][0], 128]] + dims)

    acc_ts, acc_qs = [], []
    groups = {}  # (tile, name) -> list of act instructions

    with tile.TileContext(nc) as tc:

        def mkact(tile_i, group, *args, **kw):
            ins = nc.scalar.activation(*args, **kw)
            if group is not None:
                groups.setdefault((tile_i, group), []).append(ins)
            return ins

        with (
            tc.tile_pool(name="inp", bufs=2) as pin,
            tc.tile_pool(name="rowp", bufs=2) as prow,
            tc.tile_pool(name="scr", bufs=1) as pscr,
            tc.tile_pool(name="accp", bufs=8) as pacc,
        ):
            state = {}

            def row_phase(t):
                # ---- DMA: component-major planes, all contiguous ----
                tv = pin.tile([128, 3 * R], f32, tag="tv")    # logq comps
                nc.sync.dma_start(
                    tv[:].rearrange("p (c r) -> p c r", c=3),
                    AP(pred_h, 3 * PL_P + t * C, [[D, 128], [PL_P, 3], [1, R]]))
                tt = pin.tile([128, 3 * R], f32, tag="tt")    # t comps
                nc.sync.dma_start(
                    tt[:].rearrange("p (c r) -> p c r", c=3),
                    AP(pred_h, t * C, [[D, 128], [PL_P, 3], [1, R]]))
                gtt = pin.tile([128, 3 * C], f32, tag="gtt")  # targ t comps
                nc.sync.dma_start(
                    gtt[:].rearrange("p (c r) -> p c r", c=3),
                    AP(targ_h, t * C, [[D, 128], [PL_T, 3], [1, C]]))
                gtq = pin.tile([128, 3 * C], f32, tag="gtq")  # targ q comps
                nc.sync.dma_start(
                    gtq[:].rearrange("p (c r) -> p c r", c=3),
                    AP(targ_h, 3 * PL_T + t * C, [[D, 128], [PL_T, 3], [1, C]]))

                # ---------------- row phase ----------------
                sq = pscr.tile([128, 3 * R], f32, tag="sq")
                mkact(t, None, sq[:], tv[:], AF.Square)
                n2a = pscr.tile([128, R], f32, tag="n2a")
                nc.vector.tensor_tensor(n2a[:], sq[:, 0:R], sq[:, R:2 * R], OP.add)
                n2 = pscr.tile([128, R], f32, tag="n2")
                nc.vector.tensor_tensor(n2[:], n2a[:], sq[:, 2 * R:3 * R], OP.add)
                l = pscr.tile([128, R], f32, tag="l")
                mkact(t, 'rowLE', l[:], n2[:], AF.Ln, bias=1e-16)
                n4 = pscr.tile([128, R], f32, tag="n4")
                mkact(t, 'rowLE', n4[:], l[:], AF.Exp, bias=-LN4, scale=0.5)
                i2n = pscr.tile([128, R], bf16, tag="i2n")
                mkact(t, 'rowLE', i2n[:], l[:], AF.Exp, bias=LN4SQ2, scale=-0.5)  # 4*sqrt2/n
                s4 = pscr.tile([128, R], bf16, tag="s4")
                mkact(t, 'rowTR', s4[:], n4[:], AF.Sin)
                c4 = pscr.tile([128, R], bf16, tag="c4")
                mkact(t, 'rowTR', c4[:], n4[:], AF.Sin, bias=PI2)

                sc = pscr.tile([128, R], bf16, tag="sc")
                nc.vector.tensor_tensor(sc[:], s4[:], c4[:], OP.mult)
                s4sq = pscr.tile([128, R], bf16, tag="s4sq")
                nc.vector.tensor_tensor(s4sq[:], s4[:], s4[:], OP.mult)
                cs = pscr.tile([128, R], bf16, tag="cs")
                mkact(t, None, cs[:], s4sq[:], AF.Copy, bias=1.0, scale=-2.0)  # cos(n/2)
                sc2m = pscr.tile([128, R], bf16, tag="sc2m")
                nc.vector.tensor_tensor(sc2m[:], sc[:], sc[:], OP.mult)
                A = prow.tile([128, R], bf16, tag="A")
                mkact(t, None, A[:], sc2m[:], AF.Copy, bias=SQ2, scale=-8.0 * SQ2)
                sf = pscr.tile([128, R], bf16, tag="sf")
                nc.vector.tensor_tensor(sf[:], sc[:], cs[:], OP.mult)
                sn = pscr.tile([128, R], bf16, tag="sn")
                nc.vector.tensor_tensor(sn[:], sf[:], i2n[:], OP.mult)  # sqrt2 sin/n
                U = prow.tile([128, 5 * R], bf16, tag="U")
                nc.vector.tensor_tensor(
                    sb(U, 0, [[R, 3], [1, R]]),
                    tv[:].rearrange("p (c r) -> p c r", c=3),
                    sb(sn, 0, [[0, 3], [1, R]]), OP.mult)
                nc.vector.tensor_copy(U[:, 3 * R:5 * R], U[:, 0:2 * R])

                state[t] = (A, U, tt, gtt, gtq)

            def pair_phase(t):
                A, U, tt, gtt, gtq = state.pop(t)

                def A_at(row_off):
                    return sb(A, row_off, [[0, 3], [1, C]])

                def U_at(comp_rot, row_off):
                    return sb(U, comp_rot * R + row_off, [[R, 3], [1, C]])

                def TT_at(row_off):
                    return sb(tt, row_off, [[R, 3], [1, C]])

                # ---------------- pair phase (bf16) ----------------
                cmC = lambda tl: sb(tl, 0, [[C, 3], [1, C]])
                g1 = pscr.tile([128, 5 * C], bf16, tag="g1")
                nc.vector.tensor_tensor(cmC(g1), TT_at(1), TT_at(0), OP.subtract)
                nc.vector.tensor_copy(g1[:, 3 * C:5 * C], g1[:, 0:2 * C])

                def G1(comp_rot):
                    return sb(g1, comp_rot * C, [[C, 3], [1, C]])

                p1 = pscr.tile([128, 3 * C], bf16, tag="p1")
                nc.vector.tensor_tensor(cmC(p1), U_at(1, 0), G1(2), OP.mult)
                p2 = pscr.tile([128, 3 * C], bf16, tag="p2")
                nc.vector.tensor_tensor(cmC(p2), U_at(2, 0), G1(1), OP.mult)
                b = pscr.tile([128, 5 * C], bf16, tag="b")
                nc.vector.tensor_tensor(cmC(b), p1[:], p2[:], OP.subtract)
                nc.vector.tensor_copy(b[:, 3 * C:5 * C], b[:, 0:2 * C])

                def B(comp_rot):
                    return sb(b, comp_rot * C, [[C, 3], [1, C]])

                q1 = pscr.tile([128, 3 * C], bf16, tag="p1")
                nc.vector.tensor_tensor(cmC(q1), U_at(1, 0), B(2), OP.mult)
                q2 = pscr.tile([128, 3 * C], bf16, tag="p2")
                nc.vector.tensor_tensor(cmC(q2), U_at(2, 0), B(1), OP.mult)
                cp = pscr.tile([128, 3 * C], bf16, tag="cp")
                nc.vector.tensor_tensor(cmC(cp), q1[:], q2[:], OP.subtract)
                m = pscr.tile([128, 3 * C], bf16, tag="m")
                nc.vector.tensor_tensor(cmC(m), A_at(0), B(0), OP.mult)

                tgtb = pscr.tile([128, 3 * C], bf16, tag="tgtb")
                mkact(t, None, tgtb[:], gtt[:], AF.Copy)
                g = pscr.tile([128, 3 * C], bf16, tag="g")
                nc.vector.tensor_tensor(g[:], g1[:, 0:3 * C], tgtb[:], OP.subtract)
                gc = pscr.tile([128, 3 * C], bf16, tag="gc")
                nc.vector.tensor_tensor(gc[:], g[:], cp[:], OP.add)
                dump = pscr.tile([128, 3 * C], bf16, tag="dump")
                dfft = pscr.tile([128, 3 * C], bf16, tag="dfft")
                nc.vector.tensor_tensor(dfft[:], gc[:], m[:], OP.subtract)
                acc_t = pacc.tile([128, 1], f32, tag="acct")
                nc.vector.scalar_tensor_tensor(dump[:], dfft[:], -1.0, dfft[:],
                                               OP.mult, OP.max, accum_out=acc_t[:])
                acc_ts.append(acc_t)

                # rotation part: qV = A0*U1 - A1*U0 - U0 x U1 (= 2*qv)
                mA = pscr.tile([128, C], bf16, tag="mA")
                nc.vector.tensor_tensor(mA[:], A[:, 0:C], A[:, 1:1 + C], OP.mult)
                mU = pscr.tile([128, 3 * C], bf16, tag="p1")
                nc.vector.tensor_tensor(cmC(mU), U_at(0, 0), U_at(0, 1), OP.mult)
                s1 = pscr.tile([128, C], bf16, tag="s1")
                nc.vector.tensor_tensor(s1[:], mU[:, 0:C], mU[:, C:2 * C], OP.add)
                s2 = pscr.tile([128, C], bf16, tag="s2")
                nc.vector.tensor_tensor(s2[:], s1[:], mU[:, 2 * C:3 * C], OP.add)
                qs2 = pscr.tile([128, C], bf16, tag="qs2")
                nc.vector.tensor_tensor(qs2[:], s2[:], mA[:], OP.add)

                pA = pscr.tile([128, 3 * C], bf16, tag="p1")
                nc.vector.tensor_tensor(cmC(pA), A_at(0), U_at(0, 1), OP.mult)
                pB = pscr.tile([128, 3 * C], bf16, tag="p2")
                nc.vector.tensor_tensor(cmC(pB), A_at(1), U_at(0, 0), OP.mult)
                w1 = pscr.tile([128, 3 * C], bf16, tag="w1")
                nc.vector.tensor_tensor(w1[:], pA[:], pB[:], OP.subtract)
                c1 = pscr.tile([128, 3 * C], bf16, tag="p1")
                nc.vector.tensor_tensor(cmC(c1), U_at(1, 0), U_at(2, 1), OP.mult)
                c2 = pscr.tile([128, 3 * C], bf16, tag="p2")
                nc.vector.tensor_tensor(cmC(c2), U_at(2, 0), U_at(1, 1), OP.mult)
                cr = pscr.tile([128, 3 * C], bf16, tag="cr")
                nc.vector.tensor_tensor(cr[:], c1[:], c2[:], OP.subtract)
                qV = pscr.tile([128, 3 * C], bf16, tag="qV")
                nc.vector.tensor_tensor(qV[:], w1[:], cr[:], OP.subtract)

                qVsq = pscr.tile([128, 3 * C], bf16, tag="p1")
                mkact(t, None, qVsq[:], qV[:], AF.Square)
                nva = pscr.tile([128, C], bf16, tag="s1")
                nc.vector.tensor_tensor(nva[:], qVsq[:, 0:C], qVsq[:, C:2 * C], OP.add)
                nv2 = pscr.tile([128, C], bf16, tag="s2")
                nc.vector.tensor_tensor(nv2[:], nva[:], qVsq[:, 2 * C:3 * C], OP.add)

                lq = pscr.tile([128, C], f32, tag="lq")
                mkact(t, 'pairLE', lq[:], nv2[:], AF.Ln, bias=1e-16)
                rs = pscr.tile([128, C], bf16, tag="rs")
                mkact(t, 'pairLE', rs[:], lq[:], AF.Exp, scale=-0.5)
                r2 = pscr.tile([128, C], bf16, tag="r2")
                nc.vector.tensor_tensor(r2[:], qs2[:], rs[:], OP.mult)
                at = pscr.tile([128, C], f32, tag="at")
                mkact(t, 'pairTR', at[:], r2[:], AF.Arctan, scale=-1.0)
                atp = pscr.tile([128, C], bf16, tag="atp")
                mkact(t, None, atp[:], at[:], AF.Copy, bias=PI2)
                ratio = pscr.tile([128, C], bf16, tag="ratio")
                nc.vector.tensor_tensor(ratio[:], atp[:], rs[:], OP.mult)

                ld = pscr.tile([128, 3 * C], bf16, tag="w1")
                nc.vector.tensor_tensor(cmC(ld), cmC(qV),
                                        sb(ratio, 0, [[0, 3], [1, C]]), OP.mult)
                tgqb = pscr.tile([128, 3 * C], bf16, tag="tgtb")
                mkact(t, None, tgqb[:], gtq[:], AF.Copy)
                ldiff = pscr.tile([128, 3 * C], bf16, tag="dfft")
                nc.vector.tensor_tensor(ldiff[:], ld[:], tgqb[:], OP.subtract)
                acc_q = pacc.tile([128, 1], f32, tag="accq")
                nc.vector.scalar_tensor_tensor(dump[:], ldiff[:], -1.0, ldiff[:],
                                               OP.mult, OP.max, accum_out=acc_q[:])
                acc_qs.append(acc_q)

            for t in range(NT):
                row_phase(t)
                pair_phase(t)

            # chain LUT activations so same-table-set groups run
            # contiguously (pipelined across tiles): 8 table loads total.
            order = []
            for ti in range(NT):
                order.append(("rowLE", ti))
                if ti >= 1:
                    order.append(("pairTR", ti - 1))
                order.append(("rowTR", ti))
                order.append(("pairLE", ti))
            order.append(("pairTR", NT - 1))
            seq = []
            for gname, ti in order:
                seq.extend(groups.get((ti, gname), []))
            for i in range(1, len(seq)):
                _add_dep_helper(seq[i].ins, seq[i - 1].ins, False,
                                "act table-set grouping")

            tot = pacc.tile([128, 2], f32, tag="tot")
            tmp_t = pacc.tile([128, 1], f32, tag="tmpt")
            nc.vector.tensor_tensor(tmp_t[:], acc_ts[0][:], acc_ts[1][:], OP.add)
            nc.vector.tensor_tensor(tot[:, 0:1], tmp_t[:], acc_ts[2][:], OP.add)
            tmp_q = pacc.tile([128, 1], f32, tag="tmpq")
            nc.vector.tensor_tensor(tmp_q[:], acc_qs[0][:], acc_qs[1][:], OP.add)
            nc.vector.tensor_tensor(tot[:, 1:2], tmp_q[:], acc_qs[2][:], OP.add)
            nc.sync.dma_start(out_h[:], tot[:])

    nc.compile()
    return nc


def _get_nc():
    if "nc" not in _BUILT:
        _BUILT["nc"] = _build()
    return _BUILT["nc"]


def run_device(pred, targ, trace=False):
    """pred: (1,T,6) f32, targ: (1,T-1,6) f32 -> (sum|dt|, sum|dq|, exec_ns)"""
    from concourse.bass_utils import run_bass_kernel_spmd

    nc = _get_nc()
    p = np.asarray(pred, dtype=np.float32).reshape(-1, 6)
    g = np.asarray(targ, dtype=np.float32).reshape(-1, 6)
    n_dup = ROWS_PAD - p.shape[0]
    p_pad = np.concatenate([p, np.repeat(p[-1:], n_dup, axis=0)], axis=0)
    g_pad = np.concatenate(
        [g, np.zeros((PAIRS_PAD - g.shape[0], 6), np.float32)], axis=0)

    in_maps = []
    for c in range(N_CORES):
        s = c * PPC
        in_maps.append({
            "pred": np.ascontiguousarray(p_pad[s:s + PPC + 1].T).reshape(-1),
            "targ": np.ascontiguousarray(g_pad[s:s + PPC].T).reshape(-1),
        })
    res = run_bass_kernel_spmd(nc, in_maps, core_ids=list(range(N_CORES)),
                               trace=trace)
    psum = np.stack([res.results[i]["out"] for i in range(N_CORES)])
    st = float(psum[:, :, 0].sum(dtype=np.float64))
    sq = float(psum[:, :, 1].sum(dtype=np.float64))
    return st, sq, res.exec_time_ns


def kernel(pred, targ, srx, srq):
    trace = bool(int(os.environ.get("VO_KERNEL_TRACE", "0")))
    st, sq, _ = run_device(pred, targ, trace=trace)
    t_loss = st / (3.0 * NPAIRS)
    q_loss = sq / (3.0 * NPAIRS)
    srx_v = float(np.asarray(srx).reshape(-1)[0])
    srq_v = float(np.asarray(srq).reshape(-1)[0])
    out = (np.exp(-srx_v) * t_loss + srx_v +
           np.exp(-srq_v) * q_loss + srq_v)
    return np.array([out], dtype=np.float32)



# revision 3
# speedup vs baseline: 1.2705x; 1.2705x over previous
"""Trainium2 Bass kernel for nn_AtLocPlusCriterion_VO.

loss = exp(-srx)*mean|vo_t - tg_t| + srx + exp(-srq)*mean|vo_q - tg_q| + srq
with vo = calc_vo_logq(pred[:-1], pred[1:]) (relative SE(3) pose, log-quaternion).

Sequence-parallel across 8 NeuronCores (1-row halo per shard). Inputs are
resharded host-side into component-major (SoA) bf16 planes so every on-device
access is contiguous and VectorE runs in 2x mode throughout. Per core: 1956
pairs per SBUF partition, 2 tiles of 978. Row phase (qexp via half-angle Sin
LUT, cos via Sin(pi/2 - x)) with Ln/Exp roots in f32; pair phase (rotation by
two cross products, quaternion product, log map via arctan) in bf16 on
VectorE. Cross products are issued as component-group instructions with
negative-stride slab views (no slab replication copies). Mean-L1 reduces via
Abs activations with accum_out on ScalarE; host sums 8x[128,2].
"""
import os
import numpy as np

N_CORES = 8
T_FULL = 2_000_000
NPAIRS = T_FULL - 1          # 1_999_999
D = 1956                     # pairs per partition per core
C = 978                      # pairs per tile (2 tiles)
NT = 2
R = C + 1                    # rows per tile (halo)
R2 = R + 1                   # padded slab pitch (even)
PPC = 128 * D                # 250_368 pairs per core
PAIRS_PAD = N_CORES * PPC    # 2_002_944
ROWS_PAD = PAIRS_PAD + 1

PL = PPC + 1                 # pred plane length
PT = PPC                     # targ plane length

LN2 = float(np.log(2.0))
LN2SQ2 = float(np.log(2.0 * np.sqrt(2.0)))   # i2n carries 2*sqrt2
PI2 = float(np.pi / 2.0)
SQ2 = float(np.sqrt(2.0))

_BUILT = {}


def _patch_act_tables():
    import concourse.bacc as bacc_mod
    import concourse.hw_specs as hw

    if getattr(bacc_mod, "_vo_tables_patched", False):
        return
    orig = hw.get_activation_tables

    def steered(arch, _orig=orig):
        from concourse import mybir as _mb
        AF = _mb.ActivationFunctionType
        t = {k: set(v) for k, v in _orig(arch).items()}
        # Keep all 24 entries (act_func_set_id indexes the original list);
        # drop ln/exp/arctan from the earlier sets so the table-load pass
        # resolves them to natural_log_exp_and_others / trig_and_small.
        t.get("natural_log", set()).discard(AF.Ln)
        t.get("exp_and_others", set()).discard(AF.Exp)
        t.get("sigmoid_and_others", set()).discard(AF.Arctan)
        return t

    bacc_mod.get_activation_tables = steered
    bacc_mod._vo_tables_patched = True


def _build():
    from concourse import bacc, tile, mybir
    from concourse.ap import AP
    from concourse.bass import _add_dep_helper

    _patch_act_tables()

    f32, bf16 = mybir.dt.float32, mybir.dt.bfloat16
    OP = mybir.AluOpType
    AF = mybir.ActivationFunctionType

    nc = bacc.Bacc("TRN2", target_bir_lowering=False, debug=False,
                   num_devices=N_CORES)
    pred_h = nc.declare_dram_parameter("pred", [6 * PL], bf16, isOutput=False)
    targ_h = nc.declare_dram_parameter("targ", [6 * PT], bf16, isOutput=False)
    out_h = nc.declare_dram_parameter("out", [128, 2], f32, isOutput=True)

    for v in (1e-16, -LN2, LN2SQ2, PI2):
        v = float(v)
        if (f32, v) not in nc.const_aps.aps:
            t = nc.alloc_sbuf_tensor(f"uconst-{v}", [128, 1], f32)
            nc.gpsimd.memset(t.ap(), v)
            nc.const_aps.aps[(f32, v)] = t.ap()
    nc.all_engine_barrier()

    def sb(tile_, off, dims):
        base = tile_[:, :]
        return AP(base.tensor, base.offset + off,
                  [[base.ap.to_list()[0][0], 128]] + dims)

    accs = {}
    groups = {}  # (tile, name) -> list of act instructions

    with tile.TileContext(nc) as tc:

        def mkact(tile_i, group, *args, **kw):
            ins = nc.scalar.activation(*args, **kw)
            if group is not None:
                groups.setdefault((tile_i, group), []).append(ins)
            return ins

        with (
            tc.tile_pool(name="inp", bufs=2) as pin,
            tc.tile_pool(name="rowp", bufs=2) as prow,
            tc.tile_pool(name="scr", bufs=1) as pscr,
            tc.tile_pool(name="accp", bufs=10) as pacc,
        ):
            def do_tile(t):
                TT = nc.vector.tensor_tensor

                # ---- DMA: component-major planes, all contiguous ----
                tv = pin.tile([128, 3 * R2], bf16, tag="tv")    # logq comps
                nc.sync.dma_start(
                    sb(tv, 0, [[R2, 3], [1, R]]),
                    AP(pred_h, 3 * PL + t * C, [[D, 128], [PL, 3], [1, R]]))
                tt = pin.tile([128, 3 * R2], bf16, tag="tt")    # t comps
                nc.sync.dma_start(
                    sb(tt, 0, [[R2, 3], [1, R]]),
                    AP(pred_h, t * C, [[D, 128], [PL, 3], [1, R]]))
                gtt = pin.tile([128, 3 * C], bf16, tag="gtt")   # targ t comps
                nc.sync.dma_start(
                    sb(gtt, 0, [[C, 3], [1, C]]),
                    AP(targ_h, t * C, [[D, 128], [PT, 3], [1, C]]))
                gtq = pin.tile([128, 3 * C], bf16, tag="gtq")   # targ q comps
                nc.sync.dma_start(
                    sb(gtq, 0, [[C, 3], [1, C]]),
                    AP(targ_h, 3 * PT + t * C, [[D, 128], [PT, 3], [1, C]]))

                # AU tile: slab0 = A = sqrt2*cos(n); slabs 1-3 = U = sqrt2*qv
                AU = prow.tile([128, 4 * R2], bf16, tag="AU")

                def AUs(slab, row_off, n_slab, slab_stride=None):
                    ss = R2 if slab_stride is None else slab_stride
                    return sb(AU, slab * R2 + row_off, [[ss, n_slab], [1, C]])

                # ---------------- row phase ----------------
                sq = pscr.tile([128, 3 * R2], bf16, tag="sq")
                mkact(t, 'rowLE', sb(sq, 0, [[R2, 3], [1, R]]),
                      sb(tv, 0, [[R2, 3], [1, R]]), AF.Square)
                n2a = pscr.tile([128, R2], bf16, tag="n2a")
                TT(n2a[:, 0:R], sq[:, 0:R], sq[:, R2:R2 + R], OP.add)
                n2 = pscr.tile([128, R2], bf16, tag="n2")
                TT(n2[:, 0:R], n2a[:, 0:R], sq[:, 2 * R2:2 * R2 + R], OP.add)
                l = pscr.tile([128, R2], f32, tag="l")
                mkact(t, 'rowLE', l[:, 0:R], n2[:, 0:R], AF.Ln, bias=1e-16)
                nh = pscr.tile([128, R2], f32, tag="nh")
                mkact(t, 'rowLE', nh[:, 0:R], l[:, 0:R], AF.Exp,
                      bias=-LN2, scale=0.5)                      # n/2
                i2n = pscr.tile([128, R2], bf16, tag="i2n")
                mkact(t, 'rowLE', i2n[:, 0:R], l[:, 0:R], AF.Exp,
                      bias=LN2SQ2, scale=-0.5)                   # 2*sqrt2/n
                sh = pscr.tile([128, R2], bf16, tag="sh")
                mkact(t, 'rowTR', sh[:, 0:R], nh[:, 0:R], AF.Sin)
                ch = pscr.tile([128, R2], bf16, tag="ch")
                mkact(t, 'rowTR', ch[:, 0:R], nh[:, 0:R], AF.Sin,
                      bias=PI2, scale=-1.0)                      # cos(n/2)
                shsq = pscr.tile([128, R2], bf16, tag="shsq")
                mkact(t, 'rowTR', shsq[:, 0:R], sh[:, 0:R], AF.Square)
                # A = sqrt2*cos(n) = sqrt2 - 2*sqrt2*sin^2(n/2)
                mkact(t, 'rowTR', sb(AU, 0, [[1, R]]), shsq[:, 0:R], AF.Copy,
                      bias=SQ2, scale=-2.0 * SQ2)
                sinn = pscr.tile([128, R2], bf16, tag="sinn")
                TT(sinn[:, 0:R], sh[:, 0:R], ch[:, 0:R], OP.mult)  # sin(n)/2
                sn = pscr.tile([128, R2], bf16, tag="sn")
                TT(sn[:, 0:R], sinn[:, 0:R], i2n[:, 0:R], OP.mult)
                # U = v * sqrt2*sin(n)/n
                TT(sb(AU, R2, [[R2, 3], [1, R]]),
                   sb(tv, 0, [[R2, 3], [1, R]]),
                   sb(sn, 0, [[0, 3], [1, R]]), OP.mult)

                # ---------------- pair phase (bf16) ----------------
                cm = lambda tl: sb(tl, 0, [[C, 3], [1, C]])

                def S3(tl, slab, n_slab=1, stride=C):
                    return sb(tl, slab * C, [[stride, n_slab], [1, C]])

                d = pscr.tile([128, 3 * C], bf16, tag="d")
                TT(cm(d), sb(tt, 1, [[R2, 3], [1, C]]),
                   sb(tt, 0, [[R2, 3], [1, C]]), OP.subtract)

                def cross_pair(out_t, u_row, v_t, v_row, v_pitch, v_tile_is_AU):
                    """out_c = U_{c+1}@u_row * V_{c+2}@v_row  (c = 0,1,2)
                    minus-partner handled by caller. V addressed by slabs of
                    v_pitch in tile v_t (AU slabs are at (1+k)*R2)."""
                    def V(slab, nsl, sstride):
                        base = (1 + slab) * R2 if v_tile_is_AU else slab * v_pitch
                        ss = sstride * (R2 if v_tile_is_AU else v_pitch)
                        return sb(v_t, base + v_row, [[ss, nsl], [1, C]])
                    # c in {0,1}: U slabs {1,2}, V slabs {2,0} (stride -2)
                    TT(sb(out_t, 0, [[C, 2], [1, C]]),
                       AUs(2, u_row, 2), V(2, 2, -2), OP.mult)
                    # c = 2: U slab {0}, V slab {1}
                    TT(sb(out_t, 2 * C, [[1, C]]),
                       AUs(1, u_row, 1), V(1, 1, 1), OP.mult)

                def cross_pair2(out_t, u_row, v_t, v_row, v_pitch, v_tile_is_AU):
                    """out_c = U_{c+2}@u_row * V_{c+1}@v_row  (c = 0,1,2)"""
                    def V(slab, nsl, sstride):
                        base = (1 + slab) * R2 if v_tile_is_AU else slab * v_pitch
                        ss = sstride * (R2 if v_tile_is_AU else v_pitch)
                        return sb(v_t, base + v_row, [[ss, nsl], [1, C]])
                    # c in {0,1}: U slabs {2,0} (stride -2), V slabs {1,2}
                    TT(sb(out_t, 0, [[C, 2], [1, C]]),
                       AUs(3, u_row, 2, slab_stride=-2 * R2), V(1, 2, 1),
                       OP.mult)
                    # c = 2: U slab {1}, V slab {0}
                    TT(sb(out_t, 2 * C, [[1, C]]),
                       AUs(2, u_row, 1), V(0, 1, 1), OP.mult)

                x1 = pscr.tile([128, 3 * C], bf16, tag="x1")
                x2 = pscr.tile([128, 3 * C], bf16, tag="x2")
                b = pscr.tile([128, 3 * C], bf16, tag="b")
                cp = pscr.tile([128, 3 * C], bf16, tag="cp")
                m = pscr.tile([128, 3 * C], bf16, tag="m")
                g = pscr.tile([128, 3 * C], bf16, tag="g")
                dff = pscr.tile([128, 3 * C], bf16, tag="dff")

                # b = U0 x d  (x sqrt2)
                cross_pair(x1, 0, d, 0, C, False)
                cross_pair2(x2, 0, d, 0, C, False)
                TT(cm(b), cm(x1), cm(x2), OP.subtract)
                # cp = U0 x b = 2*qv0 x (qv0 x d)
                cross_pair(x1, 0, b, 0, C, False)
                cross_pair2(x2, 0, b, 0, C, False)
                TT(cm(cp), cm(x1), cm(x2), OP.subtract)
                # m = A0 * b = 2*qs0*(qv0 x d)
                TT(cm(m), sb(AU, 0, [[0, 3], [1, C]]), cm(b), OP.mult)
                # dfft = (d - tg) + cp - m
                TT(cm(g), cm(d), cm(gtt), OP.subtract)
                TT(cm(g), cm(g), cm(cp), OP.add)
                TT(cm(dff), cm(g), cm(m), OP.subtract)
                acc_t = pacc.tile([128, 1], f32, tag=f"acct{t}")
                dump = pscr.tile([128, 3 * C], bf16, tag="dump")
                mkact(t, 'pairTR', cm(dump), cm(dff), AF.Abs,
                      accum_out=acc_t[:])
                accs[("t", t)] = acc_t

                # rotation: qs2 = 2*qs_rel ; qV = 2*qv_rel
                P = pscr.tile([128, 4 * C], bf16, tag="P")
                TT(sb(P, 0, [[C, 4], [1, C]]),
                   sb(AU, 0, [[R2, 4], [1, C]]),
                   sb(AU, 1, [[R2, 4], [1, C]]), OP.mult)
                u = pscr.tile([128, 2 * C], bf16, tag="u")
                TT(sb(u, 0, [[C, 2], [1, C]]),
                   sb(P, 0, [[C, 2], [1, C]]),
                   sb(P, 2 * C, [[C, 2], [1, C]]), OP.add)
                qs2 = pscr.tile([128, C], bf16, tag="qs2")
                TT(qs2[:], u[:, 0:C], u[:, C:2 * C], OP.add)

                # w1 = A0*U1 - A1*U0
                TT(cm(x1), sb(AU, 0, [[0, 3], [1, C]]),
                   sb(AU, R2 + 1, [[R2, 3], [1, C]]), OP.mult)
                TT(cm(x2), sb(AU, 1, [[0, 3], [1, C]]),
                   sb(AU, R2, [[R2, 3], [1, C]]), OP.mult)
                w1 = pscr.tile([128, 3 * C], bf16, tag="w1")
                TT(cm(w1), cm(x1), cm(x2), OP.subtract)
                # cr = U0 x U1
                cross_pair(x1, 0, AU, 1, None, True)
                cross_pair2(x2, 0, AU, 1, None, True)
                qV = pscr.tile([128, 3 * C], bf16, tag="qV")
                TT(cm(qV), cm(x1), cm(x2), OP.subtract)   # qV <- cr temp
                TT(cm(qV), cm(w1), cm(qV), OP.subtract)   # qV = w1 - cr

                mkact(t, 'pairLE', cm(x1), cm(qV), AF.Square)
                nva = pscr.tile([128, C], bf16, tag="nva")
                TT(nva[:], x1[:, 0:C], x1[:, C:2 * C], OP.add)
                nv2 = pscr.tile([128, C], bf16, tag="nv2")
                TT(nv2[:], nva[:], x1[:, 2 * C:3 * C], OP.add)

                lq = pscr.tile([128, C], f32, tag="lq")
                mkact(t, 'pairLE', lq[:], nv2[:], AF.Ln, bias=1e-16)
                rs = pscr.tile([128, C], bf16, tag="rs")
                mkact(t, 'pairLE', rs[:], lq[:], AF.Exp, scale=-0.5)
                r2 = pscr.tile([128, C], bf16, tag="r2")
                TT(r2[:], qs2[:], rs[:], OP.mult)
                at = pscr.tile([128, C], bf16, tag="at")
                mkact(t, 'pairTR', at[:], r2[:], AF.Arctan, scale=-1.0)
                atp = pscr.tile([128, C], bf16, tag="atp")
                mkact(t, 'pairTR', atp[:], at[:], AF.Copy, bias=PI2)
                ratio = pscr.tile([128, C], bf16, tag="ratio")
                TT(ratio[:], atp[:], rs[:], OP.mult)

                TT(cm(x2), cm(qV), sb(ratio, 0, [[0, 3], [1, C]]), OP.mult)
                TT(cm(dff), cm(x2), cm(gtq), OP.subtract)
                acc_q = pacc.tile([128, 1], f32, tag=f"accq{t}")
                mkact(t, 'pairTR', cm(dump), cm(dff), AF.Abs,
                      accum_out=acc_q[:])
                accs[("q", t)] = acc_q

            for t in range(NT):
                do_tile(t)

            # chain LUT activations so same-table-set groups run contiguously
            # (pipelined across tiles): 6 table loads total for NT=2.
            order = [('rowLE', 0), ('rowTR', 0)]
            for ti in range(1, NT):
                order += [('rowLE', ti), ('pairLE', ti - 1),
                          ('rowTR', ti), ('pairTR', ti - 1)]
            order += [('pairLE', NT - 1), ('pairTR', NT - 1)]
            seq = []
            for gname, ti in order:
                seq.extend(groups.get((ti, gname), []))
            for i in range(1, len(seq)):
                _add_dep_helper(seq[i].ins, seq[i - 1].ins, False,
                                "act table-set grouping")

            tot = pacc.tile([128, 2], f32, tag="tot")
            nc.vector.tensor_tensor(tot[:, 0:1], accs[("t", 0)][:],
                                    accs[("t", 1)][:], OP.add)
            nc.vector.tensor_tensor(tot[:, 1:2], accs[("q", 0)][:],
                                    accs[("q", 1)][:], OP.add)
            nc.sync.dma_start(out_h[:], tot[:])

    nc.compile()
    return nc


def _get_nc():
    if "nc" not in _BUILT:
        _BUILT["nc"] = _build()
    return _BUILT["nc"]


def run_device(pred, targ, trace=False):
    """pred: (1,T,6) f32, targ: (1,T-1,6) f32 -> (sum|dt|, sum|dq|, exec_ns)"""
    import ml_dtypes
    from concourse.bass_utils import run_bass_kernel_spmd

    bf16 = ml_dtypes.bfloat16
    nc = _get_nc()
    p = np.asarray(pred, dtype=np.float32).reshape(-1, 6)
    g = np.asarray(targ, dtype=np.float32).reshape(-1, 6)
    n_dup = ROWS_PAD - p.shape[0]
    p_pad = np.concatenate([p, np.repeat(p[-1:], n_dup, axis=0)], axis=0)
    g_pad = np.concatenate(
        [g, np.zeros((PAIRS_PAD - g.shape[0], 6), np.float32)], axis=0)
    p_pad = p_pad.astype(bf16)
    g_pad = g_pad.astype(bf16)

    in_maps = []
    for c in range(N_CORES):
        s = c * PPC
        in_maps.append({
            "pred": np.ascontiguousarray(p_pad[s:s + PPC + 1].T).reshape(-1),
            "targ": np.ascontiguousarray(g_pad[s:s + PPC].T).reshape(-1),
        })
    res = run_bass_kernel_spmd(nc, in_maps, core_ids=list(range(N_CORES)),
                               trace=trace)
    psum = np.stack([np.asarray(res.results[i]["out"], dtype=np.float64)
                     for i in range(N_CORES)])
    st = float(psum[:, :, 0].sum())
    sq = float(psum[:, :, 1].sum())
    return st, sq, res.exec_time_ns


def kernel(pred, targ, srx, srq):
    trace = bool(int(os.environ.get("VO_KERNEL_TRACE", "0")))
    st, sq, _ = run_device(pred, targ, trace=trace)
    t_loss = st / (3.0 * NPAIRS)
    q_loss = sq / (3.0 * NPAIRS)
    srx_v = float(np.asarray(srx).reshape(-1)[0])
    srq_v = float(np.asarray(srq).reshape(-1)[0])
    out = (np.exp(-srx_v) * t_loss + srx_v +
           np.exp(-srq_v) * q_loss + srq_v)
    return np.array([out], dtype=np.float32)


# revision 13
# speedup vs baseline: 1.3480x; 1.0610x over previous
"""Trainium2 Bass kernel for nn_AtLocPlusCriterion_VO.

loss = exp(-srx)*mean|vo_t - tg_t| + srx + exp(-srq)*mean|vo_q - tg_q| + srq
with vo = calc_vo_logq(pred[:-1], pred[1:]) (relative SE(3) pose, log-quaternion).

Sequence-parallel across 8 NeuronCores (1-row halo per shard). Inputs are
resharded host-side into component-major (SoA) bf16 planes so every on-device
access is contiguous and VectorE runs in 2x mode throughout. Per core: 1956
pairs per SBUF partition, 2 tiles of 978. Row phase (qexp via half-angle Sin
LUT, cos via Sin(pi/2 - x)) with Ln/Exp roots in f32; pair phase (rotation by
two cross products, quaternion product, log map via arctan) in bf16 on
VectorE, ordered rotation-first so the arctan LUT chain overlaps the
translation math. |qv_rel|^2 runs on GpSimd. Cross products are issued as
component-group instructions with negative-stride slab views (no slab
replication copies). Mean-L1 reduces via Abs activations with accum_out on
ScalarE; host sums 8x[128,2].
"""
import os
import numpy as np

N_CORES = 8
T_FULL = 2_000_000
NPAIRS = T_FULL - 1          # 1_999_999
D = 1956                     # pairs per partition per core
C = 978                      # pairs per tile (2 tiles)
NT = 2
R = C + 1                    # rows per tile (halo)
R2 = R + 1                   # padded slab pitch (even)
PPC = 128 * D                # 250_368 pairs per core
PAIRS_PAD = N_CORES * PPC    # 2_002_944
ROWS_PAD = PAIRS_PAD + 1

PL = PPC + 1                 # pred plane length
PT = PPC                     # targ plane length

LN2 = float(np.log(2.0))
LN2SQ2 = float(np.log(2.0 * np.sqrt(2.0)))   # i2n carries 2*sqrt2
PI2 = float(np.pi / 2.0)
SQ2 = float(np.sqrt(2.0))

_BUILT = {}


def _patch_act_tables():
    import concourse.bacc as bacc_mod
    import concourse.hw_specs as hw

    if getattr(bacc_mod, "_vo_tables_patched", False):
        return
    orig = hw.get_activation_tables

    def steered(arch, _orig=orig):
        from concourse import mybir as _mb
        AF = _mb.ActivationFunctionType
        t = {k: set(v) for k, v in _orig(arch).items()}
        # Keep all 24 entries (act_func_set_id indexes the original list);
        # drop ln/exp/arctan from the earlier sets so the table-load pass
        # resolves them to natural_log_exp_and_others / trig_and_small.
        t.get("natural_log", set()).discard(AF.Ln)
        t.get("exp_and_others", set()).discard(AF.Exp)
        t.get("sigmoid_and_others", set()).discard(AF.Arctan)
        return t

    bacc_mod.get_activation_tables = steered
    bacc_mod._vo_tables_patched = True


def _build():
    from concourse import bacc, tile, mybir
    from concourse.ap import AP
    from concourse.bass import _add_dep_helper

    _patch_act_tables()

    f32, bf16 = mybir.dt.float32, mybir.dt.bfloat16
    OP = mybir.AluOpType
    AF = mybir.ActivationFunctionType

    nc = bacc.Bacc("TRN2", target_bir_lowering=False, debug=False,
                   num_devices=N_CORES)
    pred_h = nc.declare_dram_parameter("pred", [6 * PL], bf16, isOutput=False)
    targ_h = nc.declare_dram_parameter("targ", [6 * PT], bf16, isOutput=False)
    out_h = nc.declare_dram_parameter("out", [128, 2], f32, isOutput=True)

    for v in (1e-16, -LN2, LN2SQ2, PI2):
        v = float(v)
        if (f32, v) not in nc.const_aps.aps:
            t = nc.alloc_sbuf_tensor(f"uconst-{v}", [128, 1], f32)
            nc.gpsimd.memset(t.ap(), v)
            nc.const_aps.aps[(f32, v)] = t.ap()
    nc.all_engine_barrier()

    def sb(tile_, off, dims):
        base = tile_[:, :]
        return AP(base.tensor, base.offset + off,
                  [[base.ap.to_list()[0][0], 128]] + dims)

    accs = {}
    groups = {}  # (tile, name) -> list of act instructions

    with tile.TileContext(nc) as tc:

        def mkact(tile_i, group, *args, **kw):
            ins = nc.scalar.activation(*args, **kw)
            if group is not None:
                groups.setdefault((tile_i, group), []).append(ins)
            return ins

        with (
            tc.tile_pool(name="inp", bufs=2) as pin,
            tc.tile_pool(name="rowp", bufs=2) as prow,
            tc.tile_pool(name="scr", bufs=1) as pscr,
            tc.tile_pool(name="accp", bufs=10) as pacc,
        ):
            TT = nc.vector.tensor_tensor
            GT = nc.gpsimd.tensor_tensor
            state = {}

            def dma_tile(t):
                tv = pin.tile([128, 3 * R2], bf16, tag="tv")    # logq comps
                nc.sync.dma_start(
                    sb(tv, 0, [[R2, 3], [1, R]]),
                    AP(pred_h, 3 * PL + t * C, [[D, 128], [PL, 3], [1, R]]))
                tt = pin.tile([128, 3 * R2], bf16, tag="tt")    # t comps
                nc.sync.dma_start(
                    sb(tt, 0, [[R2, 3], [1, R]]),
                    AP(pred_h, t * C, [[D, 128], [PL, 3], [1, R]]))
                gtt = pin.tile([128, 3 * C], bf16, tag="gtt")   # targ t comps
                nc.sync.dma_start(
                    sb(gtt, 0, [[C, 3], [1, C]]),
                    AP(targ_h, t * C, [[D, 128], [PT, 3], [1, C]]))
                gtq = pin.tile([128, 3 * C], bf16, tag="gtq")   # targ q comps
                nc.sync.dma_start(
                    sb(gtq, 0, [[C, 3], [1, C]]),
                    AP(targ_h, 3 * PT + t * C, [[D, 128], [PT, 3], [1, C]]))
                state[t] = (tv, tt, gtt, gtq)

            def row_acts(t):
                """ScalarE parts of the row phase (LUT chain)."""
                tv, tt, gtt, gtq = state[t]
                sq = pscr.tile([128, 3 * R2], bf16, tag="sq")
                mkact(t, 'rowLE', sb(sq, 0, [[R2, 3], [1, R]]),
                      sb(tv, 0, [[R2, 3], [1, R]]), AF.Square)
                state[(t, 'sq')] = sq

            def row_acts2(t):
                l, nh = state[(t, 'l')], state[(t, 'nh')]
                i2n = pscr.tile([128, R2], bf16, tag="i2n")
                mkact(t, 'rowLE', i2n[:, 0:R], l[:, 0:R], AF.Exp,
                      bias=LN2SQ2, scale=-0.5)                   # 2*sqrt2/n
                sh = pscr.tile([128, R2], bf16, tag="sh")
                mkact(t, 'rowTR', sh[:, 0:R], nh[:, 0:R], AF.Sin)
                ch = pscr.tile([128, R2], bf16, tag="ch")
                mkact(t, 'rowTR', ch[:, 0:R], nh[:, 0:R], AF.Sin,
                      bias=PI2, scale=-1.0)                      # cos(n/2)
                shsq = pscr.tile([128, R2], bf16, tag="shsq")
                mkact(t, 'rowTR', shsq[:, 0:R], sh[:, 0:R], AF.Square)
                state[(t, 'i2n')], state[(t, 'sh')], state[(t, 'ch')] = \
                    i2n, sh, ch
                state[(t, 'shsq')] = shsq

            def row_vec_a(t):
                """V: n2 chain + S: l, nh (issued here to sit between sq and
                the sin calls in the act chain)."""
                sq = state[(t, 'sq')]
                n2a = pscr.tile([128, R2], bf16, tag="n2a")
                TT(n2a[:, 0:R], sq[:, 0:R], sq[:, R2:R2 + R], OP.add)
                n2 = pscr.tile([128, R2], bf16, tag="n2")
                TT(n2[:, 0:R], n2a[:, 0:R], sq[:, 2 * R2:2 * R2 + R], OP.add)
                l = pscr.tile([128, R2], f32, tag="l")
                mkact(t, 'rowLE', l[:, 0:R], n2[:, 0:R], AF.Ln, bias=1e-16)
                nh = pscr.tile([128, R2], f32, tag="nh")
                mkact(t, 'rowLE', nh[:, 0:R], l[:, 0:R], AF.Exp,
                      bias=-LN2, scale=0.5)                      # n/2
                state[(t, 'l')], state[(t, 'nh')] = l, nh

            def row_vec_b(t):
                tv = state[t][0]
                sh, ch = state[(t, 'sh')], state[(t, 'ch')]
                shsq, i2n = state[(t, 'shsq')], state[(t, 'i2n')]
                AU = prow.tile([128, 4 * R2], bf16, tag="AU")
                # A = sqrt2*cos(n) = sqrt2 - 2*sqrt2*sin^2(n/2)
                mkact(t, 'rowTR', sb(AU, 0, [[1, R]]), shsq[:, 0:R], AF.Copy,
                      bias=SQ2, scale=-2.0 * SQ2)
                sinn = pscr.tile([128, R2], bf16, tag="sinn")
                TT(sinn[:, 0:R], sh[:, 0:R], ch[:, 0:R], OP.mult)  # sin(n)/2
                sn = pscr.tile([128, R2], bf16, tag="sn")
                TT(sn[:, 0:R], sinn[:, 0:R], i2n[:, 0:R], OP.mult)
                # U = v * sqrt2*sin(n)/n
                TT(sb(AU, R2, [[R2, 3], [1, R]]),
                   sb(tv, 0, [[R2, 3], [1, R]]),
                   sb(sn, 0, [[0, 3], [1, R]]), OP.mult)
                state[(t, 'AU')] = AU

            def pair_d(t):
                tt = state[t][1]
                d = pscr.tile([128, 3 * C], bf16, tag="d")
                TT(sb(d, 0, [[C, 3], [1, C]]),
                   sb(tt, 1, [[R2, 3], [1, C]]),
                   sb(tt, 0, [[R2, 3], [1, C]]), OP.subtract)
                state[(t, 'd')] = d

            def pair_rot(t):
                tv, tt, gtt, gtq = state[t]
                AU = state[(t, 'AU')]
                d = state[(t, 'd')]
                cm = lambda tl: sb(tl, 0, [[C, 3], [1, C]])

                def AUs(slab, row_off, n_slab, slab_stride=None):
                    ss = R2 if slab_stride is None else slab_stride
                    return sb(AU, slab * R2 + row_off, [[ss, n_slab], [1, C]])

                def cross_pair(out_t, u_row, v_t, v_row, v_pitch, v_is_AU):
                    """out_c = U_{c+1}@u_row * V_{c+2}@v_row  (c = 0,1,2)"""
                    def V(slab, nsl, sstride):
                        base = (1 + slab) * R2 if v_is_AU else slab * v_pitch
                        ss = sstride * (R2 if v_is_AU else v_pitch)
                        return sb(v_t, base + v_row, [[ss, nsl], [1, C]])
                    TT(sb(out_t, 0, [[C, 2], [1, C]]),
                       AUs(2, u_row, 2), V(2, 2, -2), OP.mult)
                    TT(sb(out_t, 2 * C, [[1, C]]),
                       AUs(1, u_row, 1), V(1, 1, 1), OP.mult)

                def cross_pair2(out_t, u_row, v_t, v_row, v_pitch, v_is_AU):
                    """out_c = U_{c+2}@u_row * V_{c+1}@v_row  (c = 0,1,2)"""
                    def V(slab, nsl, sstride):
                        base = (1 + slab) * R2 if v_is_AU else slab * v_pitch
                        ss = sstride * (R2 if v_is_AU else v_pitch)
                        return sb(v_t, base + v_row, [[ss, nsl], [1, C]])
                    TT(sb(out_t, 0, [[C, 2], [1, C]]),
                       AUs(3, u_row, 2, slab_stride=-2 * R2), V(1, 2, 1),
                       OP.mult)
                    TT(sb(out_t, 2 * C, [[1, C]]),
                       AUs(2, u_row, 1), V(0, 1, 1), OP.mult)

                x1 = pscr.tile([128, 3 * C], bf16, tag="x1")
                x2 = pscr.tile([128, 3 * C], bf16, tag="x2")

                # ---- rotation products first: qs2 = 2*qs_rel, qV = 2*qv_rel
                P = pscr.tile([128, 4 * C], bf16, tag="P")
                TT(sb(P, 0, [[C, 4], [1, C]]),
                   sb(AU, 0, [[R2, 4], [1, C]]),
                   sb(AU, 1, [[R2, 4], [1, C]]), OP.mult)
                u = pscr.tile([128, 2 * C], bf16, tag="u")
                TT(sb(u, 0, [[C, 2], [1, C]]),
                   sb(P, 0, [[C, 2], [1, C]]),
                   sb(P, 2 * C, [[C, 2], [1, C]]), OP.add)
                qs2 = pscr.tile([128, C], bf16, tag="qs2")
                TT(qs2[:], u[:, 0:C], u[:, C:2 * C], OP.add)

                # w1 = A0*U1 - A1*U0
                TT(cm(x1), sb(AU, 0, [[0, 3], [1, C]]),
                   sb(AU, R2 + 1, [[R2, 3], [1, C]]), OP.mult)
                TT(cm(x2), sb(AU, 1, [[0, 3], [1, C]]),
                   sb(AU, R2, [[R2, 3], [1, C]]), OP.mult)
                w1 = pscr.tile([128, 3 * C], bf16, tag="w1")
                TT(cm(w1), cm(x1), cm(x2), OP.subtract)
                # cr = U0 x U1 ; qV = w1 - cr
                cross_pair(x1, 0, AU, 1, None, True)
                cross_pair2(x2, 0, AU, 1, None, True)
                qV = pscr.tile([128, 3 * C], bf16, tag="qV")
                TT(cm(qV), cm(x1), cm(x2), OP.subtract)   # qV <- cr temp
                TT(cm(qV), cm(w1), cm(qV), OP.subtract)   # qV = w1 - cr

                # |qV|^2 square on GpSimd (overlaps V translation below)
                qsq = pscr.tile([128, 3 * C], bf16, tag="qsq")
                GT(cm(qsq), cm(qV), cm(qV), OP.mult)
                state[(t, 'rot')] = (qV, qsq, qs2)

            def pair_trans(t):
                tv, tt, gtt, gtq = state.pop(t)
                AU = state.pop((t, 'AU'))
                d = state.pop((t, 'd'))
                qV, qsq, qs2 = state.pop((t, 'rot'))
                cm = lambda tl: sb(tl, 0, [[C, 3], [1, C]])

                def AUs(slab, row_off, n_slab, slab_stride=None):
                    ss = R2 if slab_stride is None else slab_stride
                    return sb(AU, slab * R2 + row_off, [[ss, n_slab], [1, C]])

                def cross_pair(out_t, u_row, v_t, v_row, v_pitch, v_is_AU):
                    """out_c = U_{c+1}@u_row * V_{c+2}@v_row  (c = 0,1,2)"""
                    def V(slab, nsl, sstride):
                        base = (1 + slab) * R2 if v_is_AU else slab * v_pitch
                        ss = sstride * (R2 if v_is_AU else v_pitch)
                        return sb(v_t, base + v_row, [[ss, nsl], [1, C]])
                    TT(sb(out_t, 0, [[C, 2], [1, C]]),
                       AUs(2, u_row, 2), V(2, 2, -2), OP.mult)
                    TT(sb(out_t, 2 * C, [[1, C]]),
                       AUs(1, u_row, 1), V(1, 1, 1), OP.mult)

                def cross_pair2(out_t, u_row, v_t, v_row, v_pitch, v_is_AU):
                    """out_c = U_{c+2}@u_row * V_{c+1}@v_row  (c = 0,1,2)"""
                    def V(slab, nsl, sstride):
                        base = (1 + slab) * R2 if v_is_AU else slab * v_pitch
                        ss = sstride * (R2 if v_is_AU else v_pitch)
                        return sb(v_t, base + v_row, [[ss, nsl], [1, C]])
                    TT(sb(out_t, 0, [[C, 2], [1, C]]),
                       AUs(3, u_row, 2, slab_stride=-2 * R2), V(1, 2, 1),
                       OP.mult)
                    TT(sb(out_t, 2 * C, [[1, C]]),
                       AUs(2, u_row, 1), V(0, 1, 1), OP.mult)

                x1 = pscr.tile([128, 3 * C], bf16, tag="x1")
                x2 = pscr.tile([128, 3 * C], bf16, tag="x2")

                # ---- translation (overlaps the LUT chain)
                b = pscr.tile([128, 3 * C], bf16, tag="b")
                cp = pscr.tile([128, 3 * C], bf16, tag="cp")
                m = pscr.tile([128, 3 * C], bf16, tag="m")
                g = pscr.tile([128, 3 * C], bf16, tag="g")
                dff = pscr.tile([128, 3 * C], bf16, tag="dff")

                cross_pair(x1, 0, d, 0, C, False)
                cross_pair2(x2, 0, d, 0, C, False)
                TT(cm(b), cm(x1), cm(x2), OP.subtract)    # b = U0 x d
                cross_pair(x1, 0, b, 0, C, False)
                cross_pair2(x2, 0, b, 0, C, False)
                TT(cm(cp), cm(x1), cm(x2), OP.subtract)   # cp = U0 x b
                # |qV|^2 sums on V here, by which point GpSimd's qsq is done
                nva = pscr.tile([128, C], bf16, tag="nva")
                TT(nva[:], qsq[:, 0:C], qsq[:, C:2 * C], OP.add)
                nv2 = pscr.tile([128, C], bf16, tag="nv2")
                TT(nv2[:], nva[:], qsq[:, 2 * C:3 * C], OP.add)
                lq = pscr.tile([128, C], f32, tag="lq")
                mkact(t, 'pairLE', lq[:], nv2[:], AF.Ln, bias=1e-16)
                rs = pscr.tile([128, C], bf16, tag="rs")
                mkact(t, 'pairLE', rs[:], lq[:], AF.Exp, scale=-0.5)
                TT(cm(m), sb(AU, 0, [[0, 3], [1, C]]), cm(b), OP.mult)
                r2 = pscr.tile([128, C], bf16, tag="r2")
                TT(r2[:], qs2[:], rs[:], OP.mult)
                at = pscr.tile([128, C], bf16, tag="at")
                mkact(t, 'pairTRa', at[:], r2[:], AF.Arctan, scale=-1.0)
                atp = pscr.tile([128, C], bf16, tag="atp")
                mkact(t, 'pairTRa', atp[:], at[:], AF.Copy, bias=PI2)
                TT(cm(g), cm(d), cm(gtt), OP.subtract)
                TT(cm(g), cm(g), cm(cp), OP.add)
                TT(cm(dff), cm(g), cm(m), OP.subtract)
                acc_t = pacc.tile([128, 1], f32, tag=f"acct{t}")
                dump = pscr.tile([128, 3 * C], bf16, tag="dump")
                mkact(t, 'pairTRb', cm(dump), cm(dff), AF.Abs,
                      accum_out=acc_t[:])
                accs[("t", t)] = acc_t
                state[(t, 'tail')] = (qV, atp, rs, gtq, x2, dff, dump)

            def pair_tail(t):
                qV, atp, rs, gtq, x2, dff, dump = state.pop((t, 'tail'))
                cm = lambda tl: sb(tl, 0, [[C, 3], [1, C]])
                ratio = pscr.tile([128, C], bf16, tag="ratio")
                TT(ratio[:], atp[:], rs[:], OP.mult)
                TT(cm(x2), cm(qV), sb(ratio, 0, [[0, 3], [1, C]]), OP.mult)
                TT(cm(dff), cm(x2), cm(gtq), OP.subtract)
                acc_q = pacc.tile([128, 1], f32, tag=f"accq{t}")
                mkact(t, 'pairTRb', cm(dump), cm(dff), AF.Abs,
                      accum_out=acc_q[:])
                accs[("q", t)] = acc_q

            # ---- schedule: software-pipelined over tiles ----
            dma_tile(0)
            row_acts(0)
            row_vec_a(0)
            row_acts2(0)
            row_vec_b(0)
            pair_d(0)
            for t in range(NT):
                if t + 1 < NT:
                    dma_tile(t + 1)
                    row_acts(t + 1)
                pair_rot(t)
                if t + 1 < NT:
                    row_vec_a(t + 1)
                pair_trans(t)
                if t + 1 < NT:
                    pair_d(t + 1)
                pair_tail(t)
                if t + 1 < NT:
                    row_acts2(t + 1)
                    row_vec_b(t + 1)

            # chain LUT activations so same-table-set groups run contiguously
            # across tiles: 6 table loads total for NT=2.
            order = [('rowLE', 0), ('rowTR', 0)]
            for t in range(NT - 1):
                order += [('rowLE', t + 1), ('pairLE', t),
                          ('pairTRa', t), ('rowTR', t + 1), ('pairTRb', t)]
            order += [('pairLE', NT - 1), ('pairTRa', NT - 1),
                      ('pairTRb', NT - 1)]
            seq = []
            for gname, ti in order:
                seq.extend(groups.get((ti, gname), []))
            for i in range(1, len(seq)):
                _add_dep_helper(seq[i].ins, seq[i - 1].ins, False,
                                "act table-set grouping")

            tot = pacc.tile([128, 2], f32, tag="tot")
            nc.vector.tensor_tensor(tot[:, 0:1], accs[("t", 0)][:],
                                    accs[("t", 1)][:], OP.add)
            nc.vector.tensor_tensor(tot[:, 1:2], accs[("q", 0)][:],
                                    accs[("q", 1)][:], OP.add)
            nc.sync.dma_start(out_h[:], tot[:])

    nc.compile()
    return nc


def _get_nc():
    if "nc" not in _BUILT:
        _BUILT["nc"] = _build()
    return _BUILT["nc"]


def run_device(pred, targ, trace=False):
    """pred: (1,T,6) f32, targ: (1,T-1,6) f32 -> (sum|dt|, sum|dq|, exec_ns)"""
    import ml_dtypes
    from concourse.bass_utils import run_bass_kernel_spmd

    bf16 = ml_dtypes.bfloat16
    nc = _get_nc()
    p = np.asarray(pred, dtype=np.float32).reshape(-1, 6)
    g = np.asarray(targ, dtype=np.float32).reshape(-1, 6)
    n_dup = ROWS_PAD - p.shape[0]
    p_pad = np.concatenate([p, np.repeat(p[-1:], n_dup, axis=0)], axis=0)
    g_pad = np.concatenate(
        [g, np.zeros((PAIRS_PAD - g.shape[0], 6), np.float32)], axis=0)
    p_pad = p_pad.astype(bf16)
    g_pad = g_pad.astype(bf16)

    in_maps = []
    for c in range(N_CORES):
        s = c * PPC
        in_maps.append({
            "pred": np.ascontiguousarray(p_pad[s:s + PPC + 1].T).reshape(-1),
            "targ": np.ascontiguousarray(g_pad[s:s + PPC].T).reshape(-1),
        })
    res = run_bass_kernel_spmd(nc, in_maps, core_ids=list(range(N_CORES)),
                               trace=trace)
    psum = np.stack([np.asarray(res.results[i]["out"], dtype=np.float64)
                     for i in range(N_CORES)])
    st = float(psum[:, :, 0].sum())
    sq = float(psum[:, :, 1].sum())
    return st, sq, res.exec_time_ns


def kernel(pred, targ, srx, srq):
    trace = bool(int(os.environ.get("VO_KERNEL_TRACE", "0")))
    st, sq, _ = run_device(pred, targ, trace=trace)
    t_loss = st / (3.0 * NPAIRS)
    q_loss = sq / (3.0 * NPAIRS)
    srx_v = float(np.asarray(srx).reshape(-1)[0])
    srq_v = float(np.asarray(srq).reshape(-1)[0])
    out = (np.exp(-srx_v) * t_loss + srx_v +
           np.exp(-srq_v) * q_loss + srq_v)
    return np.array([out], dtype=np.float32)


# revision 22
# speedup vs baseline: 1.5093x; 1.1197x over previous
"""Trainium2 Bass kernel for nn_AtLocPlusCriterion_VO.

loss = exp(-srx)*mean|vo_t - tg_t| + srx + exp(-srq)*mean|vo_q - tg_q| + srq
with vo = calc_vo_logq(pred[:-1], pred[1:]) (relative SE(3) pose, log-quaternion).

Sequence-parallel across 8 NeuronCores (1-row halo per shard). Inputs are
resharded host-side into component-major (SoA) bf16 planes so every on-device
access is contiguous and VectorE runs in 2x mode throughout. Per core: 1956
pairs per SBUF partition, 2 tiles of 978. Row phase (qexp via half-angle Sin
LUT, cos via Sin(pi/2 - x)) with Ln/Exp roots in f32; pair phase (rotation by
two cross products, quaternion product, log map via arctan) in bf16 on
VectorE, ordered rotation-first so the arctan LUT chain overlaps the
translation math. |qv_rel|^2 runs on GpSimd. Cross products are issued as
component-group instructions with negative-stride slab views (no slab
replication copies). Mean-L1 reduces via Abs activations with accum_out on
ScalarE; host sums 8x[128,2].
"""
import os
import numpy as np

N_CORES = 8
T_FULL = 2_000_000
NPAIRS = T_FULL - 1          # 1_999_999
D = 1956                     # pairs per partition per core
C = 978                      # pairs per tile (2 tiles)
NT = 2
R = C + 1                    # rows per tile (halo)
R2 = R + 1                   # padded slab pitch (even)
PPC = 128 * D                # 250_368 pairs per core
PAIRS_PAD = N_CORES * PPC    # 2_002_944
ROWS_PAD = PAIRS_PAD + 1

PL = PPC + 1                 # pred plane length
PT = PPC                     # targ plane length

LN2 = float(np.log(2.0))
LN2SQ2 = float(np.log(2.0 * np.sqrt(2.0)))   # i2n carries 2*sqrt2
PI2 = float(np.pi / 2.0)
SQ2 = float(np.sqrt(2.0))

_BUILT = {}


def _patch_act_tables():
    import concourse.bacc as bacc_mod
    import concourse.hw_specs as hw

    if getattr(bacc_mod, "_vo_tables_patched", False):
        return
    orig = hw.get_activation_tables

    def steered(arch, _orig=orig):
        from concourse import mybir as _mb
        AF = _mb.ActivationFunctionType
        t = {k: set(v) for k, v in _orig(arch).items()}
        # Keep all 24 entries (act_func_set_id indexes the original list);
        # drop ln/exp/arctan from the earlier sets so the table-load pass
        # resolves them to natural_log_exp_and_others / trig_and_small.
        t.get("natural_log", set()).discard(AF.Ln)
        t.get("exp_and_others", set()).discard(AF.Exp)
        t.get("sigmoid_and_others", set()).discard(AF.Arctan)
        return t

    bacc_mod.get_activation_tables = steered
    bacc_mod._vo_tables_patched = True


def _build():
    from concourse import bacc, tile, mybir
    from concourse.ap import AP
    from concourse.bass import _add_dep_helper

    _patch_act_tables()

    f32, bf16 = mybir.dt.float32, mybir.dt.bfloat16
    OP = mybir.AluOpType
    AF = mybir.ActivationFunctionType

    nc = bacc.Bacc("TRN2", target_bir_lowering=False, debug=False,
                   num_devices=N_CORES)
    pred_h = nc.declare_dram_parameter("pred", [6 * PL], bf16, isOutput=False)
    targ_h = nc.declare_dram_parameter("targ", [6 * PT], bf16, isOutput=False)
    out_h = nc.declare_dram_parameter("out", [128, 2], f32, isOutput=True)

    for v in (1e-16, -LN2, LN2SQ2, PI2):
        v = float(v)
        if (f32, v) not in nc.const_aps.aps:
            t = nc.alloc_sbuf_tensor(f"uconst-{v}", [128, 1], f32)
            nc.gpsimd.memset(t.ap(), v)
            nc.const_aps.aps[(f32, v)] = t.ap()
    nc.all_engine_barrier()

    def sb(tile_, off, dims):
        base = tile_[:, :]
        return AP(base.tensor, base.offset + off,
                  [[base.ap.to_list()[0][0], 128]] + dims)

    accs = {}
    groups = {}  # (tile, name) -> list of act instructions

    with tile.TileContext(nc) as tc:

        def mkact(tile_i, group, *args, **kw):
            ins = nc.scalar.activation(*args, **kw)
            if group is not None:
                groups.setdefault((tile_i, group), []).append(ins)
            return ins

        with (
            tc.tile_pool(name="inp", bufs=2) as pin,
            tc.tile_pool(name="rowp", bufs=2) as prow,
            tc.tile_pool(name="scr", bufs=1) as pscr,
            tc.tile_pool(name="accp", bufs=10) as pacc,
        ):
            TT = nc.vector.tensor_tensor
            GT = nc.gpsimd.tensor_tensor
            state = {}

            def dma_tile(t):
                tv = pin.tile([128, 3 * R2], bf16, tag="tv")    # logq comps
                nc.sync.dma_start(
                    sb(tv, 0, [[R2, 3], [1, R]]),
                    AP(pred_h, 3 * PL + t * C, [[D, 128], [PL, 3], [1, R]]))
                tt = pin.tile([128, 3 * R2], bf16, tag="tt")    # t comps
                nc.sync.dma_start(
                    sb(tt, 0, [[R2, 3], [1, R]]),
                    AP(pred_h, t * C, [[D, 128], [PL, 3], [1, R]]))
                gtt = pin.tile([128, 3 * C], bf16, tag="gtt")   # targ t comps
                nc.sync.dma_start(
                    sb(gtt, 0, [[C, 3], [1, C]]),
                    AP(targ_h, t * C, [[D, 128], [PT, 3], [1, C]]))
                gtq = pin.tile([128, 3 * C], bf16, tag="gtq")   # targ q comps
                nc.sync.dma_start(
                    sb(gtq, 0, [[C, 3], [1, C]]),
                    AP(targ_h, 3 * PT + t * C, [[D, 128], [PT, 3], [1, C]]))
                state[t] = (tv, tt, gtt, gtq)

            def row_acts(t):
                """ScalarE parts of the row phase (LUT chain)."""
                tv, tt, gtt, gtq = state[t]
                sq = pscr.tile([128, 3 * R2], bf16, tag="sq")
                mkact(t, 'rowLEa', sb(sq, 0, [[R2, 3], [1, R]]),
                      sb(tv, 0, [[R2, 3], [1, R]]), AF.Square)
                state[(t, 'sq')] = sq

            def row_acts2(t):
                l, nh = state[(t, 'l')], state[(t, 'nh')]
                i2n = pscr.tile([128, R2], bf16, tag="i2n")
                mkact(t, 'rowLEb', i2n[:, 0:R], l[:, 0:R], AF.Exp,
                      bias=LN2SQ2, scale=-0.5)                   # 2*sqrt2/n
                sh = pscr.tile([128, R2], bf16, tag="sh")
                mkact(t, 'rowTR', sh[:, 0:R], nh[:, 0:R], AF.Sin)
                ch = pscr.tile([128, R2], bf16, tag="ch")
                mkact(t, 'rowTR', ch[:, 0:R], nh[:, 0:R], AF.Sin,
                      bias=PI2, scale=-1.0)                      # cos(n/2)
                shsq = pscr.tile([128, R2], bf16, tag="shsq")
                mkact(t, 'rowTR', shsq[:, 0:R], sh[:, 0:R], AF.Square)
                state[(t, 'i2n')], state[(t, 'sh')], state[(t, 'ch')] = \
                    i2n, sh, ch
                state[(t, 'shsq')] = shsq

            def row_vec_a(t):
                """V: n2 chain + S: l, nh (issued here to sit between sq and
                the sin calls in the act chain)."""
                sq = state[(t, 'sq')]
                n2a = pscr.tile([128, R2], bf16, tag="n2a")
                TT(n2a[:, 0:R], sq[:, 0:R], sq[:, R2:R2 + R], OP.add)
                n2 = pscr.tile([128, R2], bf16, tag="n2")
                TT(n2[:, 0:R], n2a[:, 0:R], sq[:, 2 * R2:2 * R2 + R], OP.add)
                l = pscr.tile([128, R2], f32, tag="l")
                mkact(t, 'rowLEb', l[:, 0:R], n2[:, 0:R], AF.Ln, bias=1e-16)
                nh = pscr.tile([128, R2], f32, tag="nh")
                mkact(t, 'rowLEb', nh[:, 0:R], l[:, 0:R], AF.Exp,
                      bias=-LN2, scale=0.5)                      # n/2
                state[(t, 'l')], state[(t, 'nh')] = l, nh

            def row_vec_b(t):
                tv = state[t][0]
                sh, ch = state[(t, 'sh')], state[(t, 'ch')]
                shsq, i2n = state[(t, 'shsq')], state[(t, 'i2n')]
                AU = prow.tile([128, 4 * R2], bf16, tag="AU")
                # A = sqrt2*cos(n) = sqrt2 - 2*sqrt2*sin^2(n/2)
                mkact(t, 'rowTR', sb(AU, 0, [[1, R]]), shsq[:, 0:R], AF.Copy,
                      bias=SQ2, scale=-2.0 * SQ2)
                sinn = pscr.tile([128, R2], bf16, tag="sinn")
                TT(sinn[:, 0:R], sh[:, 0:R], ch[:, 0:R], OP.mult)  # sin(n)/2
                sn = pscr.tile([128, R2], bf16, tag="sn")
                TT(sn[:, 0:R], sinn[:, 0:R], i2n[:, 0:R], OP.mult)
                # U = v * sqrt2*sin(n)/n
                TT(sb(AU, R2, [[R2, 3], [1, R]]),
                   sb(tv, 0, [[R2, 3], [1, R]]),
                   sb(sn, 0, [[0, 3], [1, R]]), OP.mult)
                state[(t, 'AU')] = AU

            def pair_d(t):
                tt = state[t][1]
                d = pscr.tile([128, 3 * C], bf16, tag="d")
                TT(sb(d, 0, [[C, 3], [1, C]]),
                   sb(tt, 1, [[R2, 3], [1, C]]),
                   sb(tt, 0, [[R2, 3], [1, C]]), OP.subtract)
                state[(t, 'd')] = d

            def cross_into(x12, AUs, v_t, v_row, v_pitch, v_is_AU):
                """x12 slabs 0-2 <- U_{c+1}@r0 * V_{c+2}@v_row,
                slabs 3-5 <- U_{c+2}@r0 * V_{c+1}@v_row  (c = 0,1,2).
                3 instructions: two c-in-{0,1} halves + fused c=2 pair."""
                def V(slab, nsl, sstride):
                    base = (1 + slab) * R2 if v_is_AU else slab * v_pitch
                    ss = sstride * (R2 if v_is_AU else v_pitch)
                    return sb(v_t, base + v_row, [[ss, nsl], [1, C]])
                TT(sb(x12, 0, [[C, 2], [1, C]]),
                   AUs(2, 0, 2), V(2, 2, -2), OP.mult)
                TT(sb(x12, 3 * C, [[C, 2], [1, C]]),
                   AUs(3, 0, 2, -2 * R2), V(1, 2, 1), OP.mult)
                # c = 2 for both halves: (U0*V1 | U1*V0)
                TT(sb(x12, 2 * C, [[3 * C, 2], [1, C]]),
                   AUs(1, 0, 2), V(1, 2, -1), OP.mult)

            def pair_rot(t):
                tv, tt, gtt, gtq = state[t]
                AU = state[(t, 'AU')]
                cm = lambda tl: sb(tl, 0, [[C, 3], [1, C]])

                def AUs(slab, row_off, n_slab, slab_stride=None):
                    ss = R2 if slab_stride is None else slab_stride
                    return sb(AU, slab * R2 + row_off, [[ss, n_slab], [1, C]])

                x12 = pscr.tile([128, 6 * C], bf16, tag="x12")

                # ---- rotation products first: qs2 = 2*qs_rel, qV = 2*qv_rel
                P = pscr.tile([128, 4 * C], bf16, tag="P")
                TT(sb(P, 0, [[C, 4], [1, C]]),
                   sb(AU, 0, [[R2, 4], [1, C]]),
                   sb(AU, 1, [[R2, 4], [1, C]]), OP.mult)
                u = pscr.tile([128, 2 * C], bf16, tag="u")
                TT(sb(u, 0, [[C, 2], [1, C]]),
                   sb(P, 0, [[C, 2], [1, C]]),
                   sb(P, 2 * C, [[C, 2], [1, C]]), OP.add)
                qs2 = pscr.tile([128, C], bf16, tag="qs2")
                TT(qs2[:], u[:, 0:C], u[:, C:2 * C], OP.add)

                # w1 = A0*U1 - A1*U0
                TT(cm(x12), sb(AU, 0, [[0, 3], [1, C]]),
                   sb(AU, R2 + 1, [[R2, 3], [1, C]]), OP.mult)
                TT(sb(x12, 3 * C, [[C, 3], [1, C]]),
                   sb(AU, 1, [[0, 3], [1, C]]),
                   sb(AU, R2, [[R2, 3], [1, C]]), OP.mult)
                w1 = pscr.tile([128, 3 * C], bf16, tag="w1")
                TT(cm(w1), cm(x12), sb(x12, 3 * C, [[C, 3], [1, C]]),
                   OP.subtract)
                # cr = U0 x U1 ; qV = w1 - cr
                cross_into(x12, AUs, AU, 1, None, True)
                qV = pscr.tile([128, 3 * C], bf16, tag="qV")
                TT(cm(qV), cm(x12), sb(x12, 3 * C, [[C, 3], [1, C]]),
                   OP.subtract)                           # qV <- cr temp
                TT(cm(qV), cm(w1), cm(qV), OP.subtract)   # qV = w1 - cr

                # |qV|^2 square on ScalarE (overlaps V translation below;
                # GpSimd would contend with VectorE for the SBUF port)
                qsq = pscr.tile([128, 3 * C], bf16, tag="qsq")
                mkact(t, 'pairLE', cm(qsq), cm(qV), AF.Square)
                state[(t, 'rot')] = (qV, qsq, qs2)

            def pair_trans(t):
                tv, tt, gtt, gtq = state.pop(t)
                AU = state.pop((t, 'AU'))
                d = state.pop((t, 'd'))
                qV, qsq, qs2 = state.pop((t, 'rot'))
                cm = lambda tl: sb(tl, 0, [[C, 3], [1, C]])

                def AUs(slab, row_off, n_slab, slab_stride=None):
                    ss = R2 if slab_stride is None else slab_stride
                    return sb(AU, slab * R2 + row_off, [[ss, n_slab], [1, C]])

                x12 = pscr.tile([128, 6 * C], bf16, tag="x12")

                def cross_into(v_t, v_row, v_pitch, v_is_AU):
                    """x12 slabs 0-2 <- U_{c+1}@r0 * V_{c+2}@v_row,
                    slabs 3-5 <- U_{c+2}@r0 * V_{c+1}@v_row  (c = 0,1,2).
                    3 instructions: the two c-in-{0,1} halves + fused c=2."""
                    def V(slab, nsl, sstride):
                        base = (1 + slab) * R2 if v_is_AU else slab * v_pitch
                        ss = sstride * (R2 if v_is_AU else v_pitch)
                        return sb(v_t, base + v_row, [[ss, nsl], [1, C]])
                    TT(sb(x12, 0, [[C, 2], [1, C]]),
                       AUs(2, 0, 2), V(2, 2, -2), OP.mult)
                    TT(sb(x12, 3 * C, [[C, 2], [1, C]]),
                       AUs(3, 0, 2, slab_stride=-2 * R2), V(1, 2, 1), OP.mult)
                    # c = 2 for both halves: (U0*V1 | U1*V0)
                    TT(sb(x12, 2 * C, [[3 * C, 2], [1, C]]),
                       AUs(1, 0, 2), V(1, 2, -1), OP.mult)

                # ---- translation (overlaps the LUT chain)
                b = pscr.tile([128, 3 * C], bf16, tag="b")
                cp = pscr.tile([128, 3 * C], bf16, tag="cp")
                m = pscr.tile([128, 3 * C], bf16, tag="m")
                g = pscr.tile([128, 3 * C], bf16, tag="g")
                dff = pscr.tile([128, 3 * C], bf16, tag="dff")

                cross_into(d, 0, C, False)
                TT(cm(b), cm(x12), sb(x12, 3 * C, [[C, 3], [1, C]]),
                   OP.subtract)                           # b = U0 x d
                cross_into(b, 0, C, False)
                TT(cm(cp), cm(x12), sb(x12, 3 * C, [[C, 3], [1, C]]),
                   OP.subtract)                           # cp = U0 x b
                # |qV|^2 sums on V here, by which point ScalarE's qsq is done
                nva = pscr.tile([128, C], bf16, tag="nva")
                TT(nva[:], qsq[:, 0:C], qsq[:, C:2 * C], OP.add)
                nv2 = pscr.tile([128, C], bf16, tag="nv2")
                TT(nv2[:], nva[:], qsq[:, 2 * C:3 * C], OP.add)
                lq = pscr.tile([128, C], f32, tag="lq")
                mkact(t, 'pairLE', lq[:], nv2[:], AF.Ln, bias=1e-16)
                rs = pscr.tile([128, C], bf16, tag="rs")
                mkact(t, 'pairLE', rs[:], lq[:], AF.Exp, scale=-0.5)
                TT(cm(m), sb(AU, 0, [[0, 3], [1, C]]), cm(b), OP.mult)
                r2 = pscr.tile([128, C], bf16, tag="r2")
                TT(r2[:], qs2[:], rs[:], OP.mult)
                at = pscr.tile([128, C], bf16, tag="at")
                mkact(t, 'pairTRa', at[:], r2[:], AF.Arctan, scale=-1.0)
                TT(cm(g), cm(d), cm(gtt), OP.subtract)
                TT(cm(g), cm(g), cm(cp), OP.add)
                TT(cm(dff), cm(g), cm(m), OP.subtract)
                acc_t = pacc.tile([128, 1], f32, tag=f"acct{t}")
                dump = pscr.tile([128, 3 * C], bf16, tag="dump")
                mkact(t, 'pairTRb', cm(dump), cm(dff), AF.Abs,
                      accum_out=acc_t[:])
                accs[("t", t)] = acc_t
                state[(t, 'tail')] = (qV, at, rs, gtq, x12, dff, dump)

            def pair_tail(t):
                qV, at, rs, gtq, x12, dff, dump = state.pop((t, 'tail'))
                cm = lambda tl: sb(tl, 0, [[C, 3], [1, C]])
                ratio = pscr.tile([128, C], bf16, tag="ratio")
                # ratio = (at + pi/2) * rs   [theta / (2m)]
                nc.vector.scalar_tensor_tensor(ratio[:], at[:], PI2, rs[:],
                                               OP.add, OP.mult)
                ld = lambda o, n: sb(x12, o, [[C, 3], [1, n]])
                df = lambda o, n: sb(dff, o, [[C, 3], [1, n]])
                gq = lambda o, n: sb(gtq, o, [[C, 3], [1, n]])
                TT(cm(x12), cm(qV), sb(ratio, 0, [[0, 3], [1, C]]), OP.mult)
                acc_q = pacc.tile([128, 1], f32, tag=f"accq{t}")
                if t < NT - 1:
                    TT(cm(dff), cm(x12), cm(gtq), OP.subtract)
                    mkact(t, 'pairTRb', cm(dump), cm(dff), AF.Abs,
                          accum_out=acc_q[:])
                    accs[("q", t)] = (acc_q,)
                else:
                    # split the final |ldiff| so ScalarE overlaps VectorE
                    h = C // 2
                    acc_q2 = pacc.tile([128, 1], f32, tag=f"accq{t}b")
                    TT(df(0, h), ld(0, h), gq(0, h), OP.subtract)
                    mkact(t, 'pairTRb', sb(dump, 0, [[C, 3], [1, h]]),
                          df(0, h), AF.Abs, accum_out=acc_q[:])
                    TT(df(h, C - h), ld(h, C - h), gq(h, C - h), OP.subtract)
                    mkact(t, 'pairTRb', sb(dump, h, [[C, 3], [1, C - h]]),
                          df(h, C - h), AF.Abs, accum_out=acc_q2[:])
                    accs[("q", t)] = (acc_q, acc_q2)

            # ---- schedule: software-pipelined over tiles ----
            # dummy act: hoists the first ACT_TABLE_LOAD off the critical
            # head (runs at t~0 instead of after the first DMA)
            dummy = pacc.tile([128, 1], f32, tag="dummy")
            mkact(-1, 'init', dummy[:], nc.const_aps.aps[(f32, PI2)], AF.Ln)
            dma_tile(0)
            row_acts(0)
            row_vec_a(0)
            row_acts2(0)
            row_vec_b(0)
            pair_d(0)
            for t in range(NT):
                if t + 1 < NT:
                    dma_tile(t + 1)
                    row_acts(t + 1)
                pair_rot(t)
                if t + 1 < NT:
                    row_vec_a(t + 1)
                pair_trans(t)
                if t + 1 < NT:
                    pair_d(t + 1)
                pair_tail(t)
                if t + 1 < NT:
                    row_acts2(t + 1)
                    row_vec_b(t + 1)

            # chain LUT activations so same-table-set groups run contiguously
            # across tiles: 6 table loads total for NT=2.
            order = [('init', -1), ('rowLEa', 0), ('rowLEb', 0), ('rowTR', 0)]
            for t in range(NT - 1):
                order += [('rowLEa', t + 1), ('pairLE', t),
                          ('rowLEb', t + 1), ('pairTRa', t),
                          ('rowTR', t + 1), ('pairTRb', t)]
            order += [('pairLE', NT - 1), ('pairTRa', NT - 1),
                      ('pairTRb', NT - 1)]
            seq = []
            for gname, ti in order:
                seq.extend(groups.get((ti, gname), []))
            for i in range(1, len(seq)):
                _add_dep_helper(seq[i].ins, seq[i - 1].ins, False,
                                "act table-set grouping")

            tot = pacc.tile([128, 2], f32, tag="tot")

            def reduce_accs(dst, lst):
                while len(lst) > 2:
                    tmp = pacc.tile([128, 1], f32, tag=f"tmp{id(lst[0])}")
                    nc.vector.tensor_tensor(tmp[:], lst[0][:], lst[1][:],
                                            OP.add)
                    lst = [tmp] + lst[2:]
                nc.vector.tensor_tensor(dst, lst[0][:], lst[1][:], OP.add)

            reduce_accs(tot[:, 0:1], [accs[("t", t_)] for t_ in range(NT)])
            reduce_accs(tot[:, 1:2],
                        [a for t_ in range(NT) for a in accs[("q", t_)]])
            nc.sync.dma_start(out_h[:], tot[:])

    nc.compile()
    return nc


def _get_nc():
    if "nc" not in _BUILT:
        _BUILT["nc"] = _build()
    return _BUILT["nc"]


def run_device(pred, targ, trace=False):
    """pred: (1,T,6) f32, targ: (1,T-1,6) f32 -> (sum|dt|, sum|dq|, exec_ns)"""
    import ml_dtypes
    from concourse.bass_utils import run_bass_kernel_spmd

    bf16 = ml_dtypes.bfloat16
    nc = _get_nc()
    p = np.asarray(pred, dtype=np.float32).reshape(-1, 6)
    g = np.asarray(targ, dtype=np.float32).reshape(-1, 6)
    n_dup = ROWS_PAD - p.shape[0]
    p_pad = np.concatenate([p, np.repeat(p[-1:], n_dup, axis=0)], axis=0)
    g_pad = np.concatenate(
        [g, np.zeros((PAIRS_PAD - g.shape[0], 6), np.float32)], axis=0)
    p_pad = p_pad.astype(bf16)
    g_pad = g_pad.astype(bf16)

    in_maps = []
    for c in range(N_CORES):
        s = c * PPC
        in_maps.append({
            "pred": np.ascontiguousarray(p_pad[s:s + PPC + 1].T).reshape(-1),
            "targ": np.ascontiguousarray(g_pad[s:s + PPC].T).reshape(-1),
        })
    res = run_bass_kernel_spmd(nc, in_maps, core_ids=list(range(N_CORES)),
                               trace=trace)
    psum = np.stack([np.asarray(res.results[i]["out"], dtype=np.float64)
                     for i in range(N_CORES)])
    st = float(psum[:, :, 0].sum())
    sq = float(psum[:, :, 1].sum())
    return st, sq, res.exec_time_ns


def kernel(pred, targ, srx, srq):
    trace = bool(int(os.environ.get("VO_KERNEL_TRACE", "0")))
    st, sq, _ = run_device(pred, targ, trace=trace)
    t_loss = st / (3.0 * NPAIRS)
    q_loss = sq / (3.0 * NPAIRS)
    srx_v = float(np.asarray(srx).reshape(-1)[0])
    srq_v = float(np.asarray(srq).reshape(-1)[0])
    out = (np.exp(-srx_v) * t_loss + srx_v +
           np.exp(-srq_v) * q_loss + srq_v)
    return np.array([out], dtype=np.float32)
